# revision 2
# baseline (speedup 1.0000x reference)
"""RWKV WKV recurrence kernel v2 for Trainium2 (8 NeuronCores).

Chunked two-level-scan formulation (chunk C=16), time-major layout:
  num_t = a_{t-1} + e^{u+k_t} v_t ; den_t = b_{t-1} + e^{u+k_t} ; out = num/den
  a_t = e^w a_{t-1} + e^{k_t} v_t   (w = -exp(time_decay) < 0, per channel)

Per chunk (phase i = t mod 16, anchor m = 7.5), with z = e^k v (A-side) / e^k
(B-side):
  zX = F⊙z with F = e^{w(m-i)}   (bounded: |w|max·8.5 ≈ 44 « 88)
  G  = strict-tri-prefix(zX) + e^{u+w}·zX + D·state_repl   (3 PE matmuls, PSUM)
  num/den = P ⊙ G with P = e^{w(i-1-m)}
Chunk summaries (Σ over each 16-chunk of zX) ride the same rhs through ones
stationaries into a PSUM summary matrix [128 chunks × channels]; a DVE scan
over chunk summaries per channel (after an xbar bf16 transpose to
channel-major) yields cross-chunk states, xbar'd back for the replication
matmul. Scans touch T/16 of the data — the full-length DVE scan (2 cyc/elem)
was the old bottleneck.

Mapping: batch (8) -> one NeuronCore each. Per core: 16 row-blocks of 128
timesteps × 2 channel panels of 1024; block-interleaved two-pass schedule so
panel p+1 loads overlap panel p compute. Elementwise in bf16 (DVE 2× mode),
fp32 for raw k and the final output. Rel err vs reference ≈ 1.2e-2.
"""

import os
import sys
from contextlib import ExitStack

import numpy as np

for _p in ("/opt/trn_rl_repo", "/root/.axon_site/_ro/trn_rl_repo"):
    if os.path.isdir(_p) and _p not in sys.path:
        sys.path.insert(0, _p)

import concourse.bacc as bacc
import concourse.mybir as mybir
import concourse.tile as tile
from concourse import dve_ops as _dve_ops
from concourse.bass_utils import run_bass_kernel_spmd
from concourse.dve_spec import Spec as _Spec, lower as _dve_lower
from concourse.dve_uop import AluOp as _AluOp, DveOpSpec as _DveOpSpec

F32 = mybir.dt.float32
BF16 = mybir.dt.bfloat16
AF = mybir.ActivationFunctionType
OP = mybir.AluOpType

B, T, H = 8, 2048, 2048
N_CORES = 8
C = 16            # chunk length
M_ANCHOR = 7.5    # phase anchor
BLK = 128         # timesteps per block
NBLK = T // BLK   # 16
NJ = T // C       # global chunks = 128
PW = 512          # panel width (channels)
NP = H // PW      # 2 panels
GPP = PW // 128   # channel groups per panel = 8
CS_W = 128 + 128 + NBLK * 128 + NBLK * 128  # M, Id, W16, R16

# Fused out = num/den in ONE DVE pass: bitwise-NOT reciprocal seed +
# minimax deg-2 polynomial + multiply. Rel err ~5.1e-5.
_DIV_C0, _DIV_C1, _DIV_C2 = -0.7071067, -0.1665221, -0.013060556


def _div_mul_ref(in0, in1, c0, c1, c2):
    in0 = np.asarray(in0, np.float32)
    in1 = np.asarray(in1, np.float32)
    n = (~in0.view(np.int32)).view(np.float32)
    s = (in0 * n).astype(np.float32)
    q = (in1 * n).astype(np.float32)
    u = (s * np.float32(c2)).astype(np.float32)
    v = (np.float32(c1) + u).astype(np.float32)
    w = (s * v).astype(np.float32)
    p = (np.float32(c0) + w).astype(np.float32)
    return (q * p).astype(np.float32)


def _register_div_mul():
    name = "WKV_DIV_MUL_ANT"
    if name in _dve_ops._SUB_OPCODE_FOR_NAME:
        return next(o for o in _dve_ops.OPS if o.name == name)
    from concourse import dve_spec as _dve_spec
    Src0, Src1 = _dve_spec.Src0, _dve_spec.Src1
    C0, C1, C2 = _dve_spec.C0, _dve_spec.C1, _dve_spec.C2
    _n = _dve_spec.Bin(_AluOp.BITWISE_NOT, Src0, Src0)
    _s = Src0 * _n
    body = (Src1 * _n) * (C0 + _s * (C1 + _s * C2))
    spec = _Spec(body=body, reference=_div_mul_ref)
    shas = {}
    for ver in ("v3", "v4"):
        try:
            uops = _dve_lower(spec, ver=ver)
        except Exception:
            continue
        shas[ver] = _DveOpSpec(name=name, opcode=0, uops=uops, rd1_en=True).sha(ver)
    op = _dve_ops.DveOp(name, spec, subdim=False, uops_sha=shas)
    row = _dve_ops._CUSTOM_DVE_ROW_BASE + len(_dve_ops.OPS)
    assert row < 0x20
    _dve_ops.OPS.append(op)
    _dve_ops._SUB_OPCODE_FOR_NAME[name] = row
    _dve_ops.CUSTOM_DVE_SPECS[name] = spec
    return op


WKV_DIV_MUL = _register_div_mul()



def host_consts(time_decay, time_first):
    """Constant tiles from the [H] decay/first vectors (all fp32 host-side)."""
    u = time_first.astype(np.float64)
    w = -np.exp(time_decay.astype(np.float64))
    i_ph = np.arange(C, dtype=np.float64)[:, None]
    reps = (BLK // C, 1)
    Ft = np.tile(np.exp(w[None, :] * (M_ANCHOR - i_ph)), reps)      # [128, H]
    E3 = np.tile(np.exp(u + w)[None, :], (BLK, 1))                  # [128, H]
    Dr = np.tile(np.exp(C * w)[None, :], (BLK, 1))                  # [128, H]
    cT = np.stack([Ft, E3, Dr], axis=0).astype(np.float32)          # [3, 128, H]
    cD = np.exp(C * w).reshape(H // 128, 128).T.astype(np.float32)  # [128, 16]
    # stationaries (lhsT layout [K=128, M=128]): strict block-diag lower-tri,
    # identity, 16 summary selectors, 16 replication selectors
    Mt = np.zeros((BLK, BLK), np.float32)
    for i_ in range(BLK):
        Mt[(i_ // C) * C:i_, i_] = 1.0
    Id = np.eye(BLK, dtype=np.float32)
    W16 = np.zeros((NBLK, BLK, BLK), np.float32)
    R16 = np.zeros((NBLK, BLK, BLK), np.float32)
    for b_ in range(NBLK):
        for p in range(BLK):
            W16[b_, p, 8 * b_ + p // C] = 1.0       # summary: chunk-sum rows
        for i_ in range(BLK):
            R16[b_, 8 * b_ + i_ // C, i_] = 1.0     # replication: state rows
    cS = np.concatenate([Mt, Id] + [W16[b_] for b_ in range(NBLK)]
                        + [R16[b_] for b_ in range(NBLK)], axis=1).astype(np.float32)
    assert cS.shape == (BLK, CS_W)
    return cT, cD, cS


def build_nc():
    nc = bacc.Bacc("TRN2", target_bir_lowering=False, debug=False)

    key = nc.dram_tensor("key", [T, H], F32, kind="ExternalInput").ap()
    value = nc.dram_tensor("value", [T, H], F32, kind="ExternalInput").ap()
    cT = nc.dram_tensor("cT", [3, BLK, H], F32, kind="ExternalInput").ap()
    cD = nc.dram_tensor("cD", [BLK, H // 128], F32, kind="ExternalInput").ap()
    cS = nc.dram_tensor("cS", [BLK, CS_W], F32, kind="ExternalInput").ap()
    out = nc.dram_tensor("out", [T, H], F32, kind="ExternalOutput").ap()

    with tile.TileContext(nc) as tc, ExitStack() as ctx:
        const = ctx.enter_context(tc.tile_pool(name="const", bufs=1))
        Ft = const.tile([BLK, H], BF16)
        E3t = const.tile([BLK, H], BF16)
        Drt = const.tile([BLK, H], BF16)
        for idx, t_ in enumerate((Ft, E3t, Drt)):
            nc.gpsimd.dma_start(t_[:], cT[idx])
        Dsc = const.tile([BLK, H // 128], BF16)
        nc.gpsimd.dma_start(Dsc[:], cD)
        St = const.tile([BLK, CS_W], BF16)
        nc.gpsimd.dma_start(St[:], cS)
        Mt = St[:, 0:128]
        Id = St[:, 128:256]
        W16 = [St[:, 256 + 128 * b_: 256 + 128 * (b_ + 1)] for b_ in range(NBLK)]
        R16 = [St[:, 256 + 128 * NBLK + 128 * b_: 256 + 128 * NBLK + 128 * (b_ + 1)]
               for b_ in range(NBLK)]

        RING = NBLK + 2
        kin = ctx.enter_context(tc.tile_pool(name="kin", bufs=3))
        ekp = ctx.enter_context(tc.tile_pool(name="ekp", bufs=4))
        zp = ctx.enter_context(tc.tile_pool(name="zp", bufs=RING))
        gbp = ctx.enter_context(tc.tile_pool(name="gbp", bufs=4))
        outp = ctx.enter_context(tc.tile_pool(name="outp", bufs=3))
        scp = ctx.enter_context(tc.tile_pool(name="scp", bufs=1))
        psum_S = ctx.enter_context(tc.tile_pool(name="psS", bufs=1, space="PSUM"))
        psum_G = ctx.enter_context(tc.tile_pool(name="psG", bufs=3, space="PSUM"))

        zA, zB, E1A, E1B, SA, SB, stT = {}, {}, {}, {}, {}, {}, {}
        kv2, og2 = {}, {}

        def pass1_block(p, b):
            hs = slice(p * PW, (p + 1) * PW)
            if b % 2 == 0:
                kf = kin.tile([BLK, 2, PW], F32, tag="k")
                vf = kin.tile([BLK, 2, PW], F32, tag="v")
                r2 = key[b * BLK:(b + 2) * BLK, hs]
                nc.sync.dma_start(kf[:], r2.rearrange("(s p) h -> p s h", p=BLK))
                r2v = value[b * BLK:(b + 2) * BLK, hs]
                nc.sync.dma_start(vf[:], r2v.rearrange("(s p) h -> p s h", p=BLK))
                kv2[p] = (kf, vf)
            kf, vf = kv2[p]
            ks, vs = kf[:, b % 2, :], vf[:, b % 2, :]
            ek = ekp.tile([BLK, PW], BF16, tag="ek")
            nc.scalar.activation(ek[:], ks, AF.Exp)
            zb = zp.tile([BLK, PW], BF16, tag="zB")
            nc.vector.tensor_mul(zb[:], Ft[:, hs], ek[:])
            za = zp.tile([BLK, PW], BF16, tag="zA")
            nc.gpsimd.tensor_mul(za[:], zb[:], vs)
            e1a = zp.tile([BLK, PW], BF16, tag="e1A")
            nc.vector.tensor_mul(e1a[:], E3t[:, hs], za[:])
            e1b = zp.tile([BLK, PW], BF16, tag="e1B")
            if b % 2 == 0:
                nc.vector.tensor_mul(e1b[:], E3t[:, hs], zb[:])
            else:
                nc.gpsimd.tensor_mul(e1b[:], E3t[:, hs], zb[:])
            zA[(p, b)], zB[(p, b)] = za, zb
            E1A[(p, b)], E1B[(p, b)] = e1a, e1b
            if b == 0:
                SA[p] = psum_S.tile([BLK, PW], F32, tag="SA", name=f"SA{p}")
                SB[p] = psum_S.tile([BLK, PW], F32, tag="SB", name=f"SB{p}")
            nc.tensor.matmul(SA[p][:], W16[b], za[:], start=(b == 0), stop=(b == NBLK - 1))
            nc.tensor.matmul(SB[p][:], W16[b], zb[:], start=(b == 0), stop=(b == NBLK - 1))

        def scan_phase(p):
            hs = slice(p * PW, (p + 1) * PW)
            for q, Sps in (("A", SA[p]), ("B", SB[p])):
                sf = scp.tile([BLK, PW], BF16, tag=f"sf{q}")
                nc.vector.tensor_mul(sf[:], Drt[:, hs], Sps[:])
                st_ = scp.tile([BLK, GPP, 128], BF16, tag=f"st{q}")
                nc.sync.dma_start_transpose(st_[:], sf[:])
                x = scp.tile([BLK, GPP, 128], BF16, tag=f"x{q}")
                for g in range(GPP):
                    gg = p * GPP + g
                    nc.vector.memset(x[:, g, 0:1], 0.0)
                    d0 = Dsc[:, gg:gg + 1].broadcast_to((BLK, 127))
                    nc.vector.tensor_tensor_scan(
                        x[:, g, 1:128], d0, st_[:, g, 0:127], 0.0, OP.mult, OP.add
                    )
                stt = scp.tile([BLK, GPP, 128], BF16, tag=f"stt{q}")
                nc.sync.dma_start_transpose(
                    stt[:], x[:].rearrange("p s h -> p (s h)")
                )
                stT[(p, q)] = stt

        def pass2_block(p, b):
            hs = slice(p * PW, (p + 1) * PW)
            za, zb = zA.pop((p, b)), zB.pop((p, b))
            e1a, e1b = E1A.pop((p, b)), E1B.pop((p, b))
            GA = psum_G.tile([BLK, PW], F32, tag="GA")
            GB = psum_G.tile([BLK, PW], F32, tag="GB")
            sta = stT[(p, "A")][:].rearrange("p s h -> p (s h)")
            stb = stT[(p, "B")][:].rearrange("p s h -> p (s h)")
            nc.tensor.matmul(GA[:], Mt, za[:], start=True, stop=False)
            nc.tensor.matmul(GB[:], Mt, zb[:], start=True, stop=False)
            nc.tensor.matmul(GA[:], Id, e1a[:], start=False, stop=False)
            nc.tensor.matmul(GB[:], Id, e1b[:], start=False, stop=False)
            nc.tensor.matmul(GA[:], R16[b], sta, start=False, stop=True)
            nc.tensor.matmul(GB[:], R16[b], stb, start=False, stop=True)
            gb = gbp.tile([BLK, PW], BF16, tag="gb")
            nc.scalar.copy(gb[:], GB[:])
            if b % 2 == 0:
                og2[p] = outp.tile([BLK, 2, PW], F32, tag="out", name=f"og{p}_{b}")
            og = og2[p]
            nc.vector._custom_dve(WKV_DIV_MUL, out=og[:, b % 2, :], in0=gb[:], in1=GA[:],
                                  s0=_DIV_C0, s1=_DIV_C1, imm2=_DIV_C2)
            if b % 2 == 1:
                r2 = out[(b - 1) * BLK:(b + 1) * BLK, hs]
                nc.scalar.dma_start(r2.rearrange("(s p) h -> p s h", p=BLK), og[:])

        for b in range(NBLK):
            pass1_block(0, b)
        scan_phase(0)
        for p in range(NP - 1):
            for b in range(NBLK):
                pass2_block(p, b)
                pass1_block(p + 1, b)
            scan_phase(p + 1)
        for b in range(NBLK):
            pass2_block(NP - 1, b)

    nc.compile()
    return nc


_cache = {}


def _get_nc():
    if "nc" not in _cache:
        _cache["nc"] = build_nc()
    return _cache["nc"]


def kernel_with_results(key, value, time_decay, time_first, trace=False):
    nc = _get_nc()
    key = np.ascontiguousarray(key, dtype=np.float32)
    value = np.ascontiguousarray(value, dtype=np.float32)
    cT, cD, cS = host_consts(
        np.asarray(time_decay, np.float32), np.asarray(time_first, np.float32)
    )
    in_maps = [
        {"key": key[i], "value": value[i], "cT": cT, "cD": cD, "cS": cS}
        for i in range(N_CORES)
    ]
    res = run_bass_kernel_spmd(nc, in_maps, list(range(N_CORES)), trace=trace)
    outv = np.stack([res.results[i]["out"] for i in range(N_CORES)], axis=0)
    return outv, res


def kernel(key, value, time_decay, time_first):
    outv, _ = kernel_with_results(key, value, time_decay, time_first)
    return outv


# revision 3
# speedup vs baseline: 1.0066x; 1.0066x over previous
"""RWKV WKV recurrence kernel v2 for Trainium2 (8 NeuronCores).

Chunked two-level-scan formulation (chunk C=16), time-major layout:
  num_t = a_{t-1} + e^{u+k_t} v_t ; den_t = b_{t-1} + e^{u+k_t} ; out = num/den
  a_t = e^w a_{t-1} + e^{k_t} v_t   (w = -exp(time_decay) < 0, per channel)

Per chunk (phase i = t mod 16, anchor m = 7.5), with z = e^k v (A-side) / e^k
(B-side):
  zX = F⊙z with F = e^{w(m-i)}   (bounded: |w|max·8.5 ≈ 44 « 88)
  G  = strict-tri-prefix(zX) + e^{u+w}·zX + D·state_repl   (3 PE matmuls, PSUM)
  num/den = P ⊙ G with P = e^{w(i-1-m)}
Chunk summaries (Σ over each 16-chunk of zX) ride the same rhs through ones
stationaries into a PSUM summary matrix [128 chunks × channels]; a DVE scan
over chunk summaries per channel (after an xbar bf16 transpose to
channel-major) yields cross-chunk states, xbar'd back for the replication
matmul. Scans touch T/16 of the data — the full-length DVE scan (2 cyc/elem)
was the old bottleneck.

Mapping: batch (8) -> one NeuronCore each. Per core: 16 row-blocks of 128
timesteps × 2 channel panels of 1024; block-interleaved two-pass schedule so
panel p+1 loads overlap panel p compute. Elementwise in bf16 (DVE 2× mode),
fp32 for raw k and the final output. Rel err vs reference ≈ 1.2e-2.
"""

import os
import sys
from contextlib import ExitStack

import numpy as np

for _p in ("/opt/trn_rl_repo", "/root/.axon_site/_ro/trn_rl_repo"):
    if os.path.isdir(_p) and _p not in sys.path:
        sys.path.insert(0, _p)

import concourse.bacc as bacc
import concourse.mybir as mybir
import concourse.tile as tile
from concourse import dve_ops as _dve_ops
from concourse.bass_utils import run_bass_kernel_spmd
from concourse.dve_spec import Spec as _Spec, lower as _dve_lower
from concourse.dve_uop import AluOp as _AluOp, DveOpSpec as _DveOpSpec

F32 = mybir.dt.float32
BF16 = mybir.dt.bfloat16
AF = mybir.ActivationFunctionType
OP = mybir.AluOpType

B, T, H = 8, 2048, 2048
N_CORES = 8
C = 16            # chunk length
M_ANCHOR = 7.5    # phase anchor
BLK = 128         # timesteps per block
NBLK = T // BLK   # 16
NJ = T // C       # global chunks = 128
PW = 512          # panel width (channels)
NP = H // PW      # 2 panels
GPP = PW // 128   # channel groups per panel = 8
CS_W = 128 + 128 + NBLK * 128 + NBLK * 128  # M, Id, W16, R16

# Fused out = num/den in ONE DVE pass: bitwise-NOT reciprocal seed +
# minimax deg-2 polynomial + multiply. Rel err ~5.1e-5.
_DIV_C0, _DIV_C1, _DIV_C2 = -0.7071067, -0.1665221, -0.013060556


def _div_mul_ref(in0, in1, c0, c1, c2):
    in0 = np.asarray(in0, np.float32)
    in1 = np.asarray(in1, np.float32)
    n = (~in0.view(np.int32)).view(np.float32)
    s = (in0 * n).astype(np.float32)
    q = (in1 * n).astype(np.float32)
    u = (s * np.float32(c2)).astype(np.float32)
    v = (np.float32(c1) + u).astype(np.float32)
    w = (s * v).astype(np.float32)
    p = (np.float32(c0) + w).astype(np.float32)
    return (q * p).astype(np.float32)


def _register_div_mul():
    name = "WKV_DIV_MUL_ANT"
    if name in _dve_ops._SUB_OPCODE_FOR_NAME:
        return next(o for o in _dve_ops.OPS if o.name == name)
    from concourse import dve_spec as _dve_spec
    Src0, Src1 = _dve_spec.Src0, _dve_spec.Src1
    C0, C1, C2 = _dve_spec.C0, _dve_spec.C1, _dve_spec.C2
    _n = _dve_spec.Bin(_AluOp.BITWISE_NOT, Src0, Src0)
    _s = Src0 * _n
    body = (Src1 * _n) * (C0 + _s * (C1 + _s * C2))
    spec = _Spec(body=body, reference=_div_mul_ref)
    shas = {}
    for ver in ("v3", "v4"):
        try:
            uops = _dve_lower(spec, ver=ver)
        except Exception:
            continue
        shas[ver] = _DveOpSpec(name=name, opcode=0, uops=uops, rd1_en=True).sha(ver)
    op = _dve_ops.DveOp(name, spec, subdim=False, uops_sha=shas)
    row = _dve_ops._CUSTOM_DVE_ROW_BASE + len(_dve_ops.OPS)
    assert row < 0x20
    _dve_ops.OPS.append(op)
    _dve_ops._SUB_OPCODE_FOR_NAME[name] = row
    _dve_ops.CUSTOM_DVE_SPECS[name] = spec
    return op


WKV_DIV_MUL = _register_div_mul()



def host_consts(time_decay, time_first):
    """Constant tiles from the [H] decay/first vectors (all fp32 host-side)."""
    u = time_first.astype(np.float64)
    w = -np.exp(time_decay.astype(np.float64))
    i_ph = np.arange(C, dtype=np.float64)[:, None]
    reps = (BLK // C, 1)
    Ft = np.tile(np.exp(w[None, :] * (M_ANCHOR - i_ph)), reps)      # [128, H]
    E3 = np.tile(np.exp(u + w)[None, :], (BLK, 1))                  # [128, H]
    Dr = np.tile(np.exp(C * w)[None, :], (BLK, 1))                  # [128, H]
    cT = np.stack([Ft, E3, Dr], axis=0).astype(np.float32)          # [3, 128, H]
    cD = np.exp(C * w).reshape(H // 128, 128).T.astype(np.float32)  # [128, 16]
    # stationaries (lhsT layout [K=128, M=128]): strict block-diag lower-tri,
    # identity, 16 summary selectors, 16 replication selectors
    Mt = np.zeros((BLK, BLK), np.float32)
    for i_ in range(BLK):
        Mt[(i_ // C) * C:i_, i_] = 1.0
    Id = np.eye(BLK, dtype=np.float32)
    W16 = np.zeros((NBLK, BLK, BLK), np.float32)
    R16 = np.zeros((NBLK, BLK, BLK), np.float32)
    for b_ in range(NBLK):
        for p in range(BLK):
            W16[b_, p, 8 * b_ + p // C] = 1.0       # summary: chunk-sum rows
        for i_ in range(BLK):
            R16[b_, 8 * b_ + i_ // C, i_] = 1.0     # replication: state rows
    cS = np.concatenate([Mt, Id] + [W16[b_] for b_ in range(NBLK)]
                        + [R16[b_] for b_ in range(NBLK)], axis=1).astype(np.float32)
    assert cS.shape == (BLK, CS_W)
    return cT, cD, cS


def build_nc():
    nc = bacc.Bacc("TRN2", target_bir_lowering=False, debug=False)

    key = nc.dram_tensor("key", [T, H], F32, kind="ExternalInput").ap()
    value = nc.dram_tensor("value", [T, H], F32, kind="ExternalInput").ap()
    cT = nc.dram_tensor("cT", [3, BLK, H], F32, kind="ExternalInput").ap()
    cD = nc.dram_tensor("cD", [BLK, H // 128], F32, kind="ExternalInput").ap()
    cS = nc.dram_tensor("cS", [BLK, CS_W], F32, kind="ExternalInput").ap()
    out = nc.dram_tensor("out", [T, H], F32, kind="ExternalOutput").ap()

    with tile.TileContext(nc) as tc, ExitStack() as ctx:
        const = ctx.enter_context(tc.tile_pool(name="const", bufs=1))
        Ft = const.tile([BLK, H], BF16)
        E3t = const.tile([BLK, H], BF16)
        Drt = const.tile([BLK, H], BF16)
        for idx, t_ in enumerate((Ft, E3t, Drt)):
            nc.gpsimd.dma_start(t_[:], cT[idx])
        Dsc = const.tile([BLK, H // 128], BF16)
        nc.gpsimd.dma_start(Dsc[:], cD)
        St = const.tile([BLK, CS_W], BF16)
        nc.gpsimd.dma_start(St[:], cS)
        Mt = St[:, 0:128]
        Id = St[:, 128:256]
        W16 = [St[:, 256 + 128 * b_: 256 + 128 * (b_ + 1)] for b_ in range(NBLK)]
        R16 = [St[:, 256 + 128 * NBLK + 128 * b_: 256 + 128 * NBLK + 128 * (b_ + 1)]
               for b_ in range(NBLK)]

        RING = NBLK + 2
        kin = ctx.enter_context(tc.tile_pool(name="kin", bufs=3))
        ekp = ctx.enter_context(tc.tile_pool(name="ekp", bufs=4))
        zp = ctx.enter_context(tc.tile_pool(name="zp", bufs=RING))
        gbp = ctx.enter_context(tc.tile_pool(name="gbp", bufs=4))
        outp = ctx.enter_context(tc.tile_pool(name="outp", bufs=3))
        scp = ctx.enter_context(tc.tile_pool(name="scp", bufs=1))
        psum_S = ctx.enter_context(tc.tile_pool(name="psS", bufs=2, space="PSUM"))
        psum_G = ctx.enter_context(tc.tile_pool(name="psG", bufs=2, space="PSUM"))

        zA, zB, E1A, E1B, SA, SB, stT = {}, {}, {}, {}, {}, {}, {}
        kv2, og2 = {}, {}

        def pass1_block(p, b):
            hs = slice(p * PW, (p + 1) * PW)
            if b % 2 == 0:
                kf = kin.tile([BLK, 2, PW], F32, tag="k")
                vf = kin.tile([BLK, 2, PW], F32, tag="v")
                r2 = key[b * BLK:(b + 2) * BLK, hs]
                nc.sync.dma_start(kf[:], r2.rearrange("(s p) h -> p s h", p=BLK))
                r2v = value[b * BLK:(b + 2) * BLK, hs]
                nc.sync.dma_start(vf[:], r2v.rearrange("(s p) h -> p s h", p=BLK))
                kv2[p] = (kf, vf)
            kf, vf = kv2[p]
            ks, vs = kf[:, b % 2, :], vf[:, b % 2, :]
            ek = ekp.tile([BLK, PW], BF16, tag="ek")
            nc.scalar.activation(ek[:], ks, AF.Exp)
            zb = zp.tile([BLK, PW], BF16, tag="zB")
            nc.vector.tensor_mul(zb[:], Ft[:, hs], ek[:])
            za = zp.tile([BLK, PW], BF16, tag="zA")
            nc.gpsimd.tensor_mul(za[:], zb[:], vs)
            e1a = zp.tile([BLK, PW], BF16, tag="e1A")
            nc.vector.tensor_mul(e1a[:], E3t[:, hs], za[:])
            e1b = zp.tile([BLK, PW], BF16, tag="e1B")
            if b % 2 == 0:
                nc.vector.tensor_mul(e1b[:], E3t[:, hs], zb[:])
            else:
                nc.gpsimd.tensor_mul(e1b[:], E3t[:, hs], zb[:])
            zA[(p, b)], zB[(p, b)] = za, zb
            E1A[(p, b)], E1B[(p, b)] = e1a, e1b
            if b == 0:
                SA[p] = psum_S.tile([BLK, PW], F32, tag="SA", name=f"SA{p}")
                SB[p] = psum_S.tile([BLK, PW], F32, tag="SB", name=f"SB{p}")
            nc.tensor.matmul(SA[p][:], W16[b], za[:], start=(b == 0), stop=(b == NBLK - 1))
            nc.tensor.matmul(SB[p][:], W16[b], zb[:], start=(b == 0), stop=(b == NBLK - 1))

        def scan_phase(p):
            hs = slice(p * PW, (p + 1) * PW)
            for q, Sps in (("A", SA[p]), ("B", SB[p])):
                sf = scp.tile([BLK, PW], BF16, tag=f"sf{q}")
                nc.vector.tensor_mul(sf[:], Drt[:, hs], Sps[:])
                st_ = scp.tile([BLK, GPP, 128], BF16, tag=f"st{q}")
                nc.sync.dma_start_transpose(st_[:], sf[:])
                x = scp.tile([BLK, GPP, 128], BF16, tag=f"x{q}")
                for g in range(GPP):
                    gg = p * GPP + g
                    nc.vector.memset(x[:, g, 0:1], 0.0)
                    d0 = Dsc[:, gg:gg + 1].broadcast_to((BLK, 127))
                    nc.vector.tensor_tensor_scan(
                        x[:, g, 1:128], d0, st_[:, g, 0:127], 0.0, OP.mult, OP.add
                    )
                stt = scp.tile([BLK, GPP, 128], BF16, tag=f"stt{q}")
                nc.sync.dma_start_transpose(
                    stt[:], x[:].rearrange("p s h -> p (s h)")
                )
                stT[(p, q)] = stt

        def pass2_block(p, b):
            hs = slice(p * PW, (p + 1) * PW)
            za, zb = zA.pop((p, b)), zB.pop((p, b))
            e1a, e1b = E1A.pop((p, b)), E1B.pop((p, b))
            GA = psum_G.tile([BLK, PW], F32, tag="GA")
            GB = psum_G.tile([BLK, PW], F32, tag="GB")
            sta = stT[(p, "A")][:].rearrange("p s h -> p (s h)")
            stb = stT[(p, "B")][:].rearrange("p s h -> p (s h)")
            nc.tensor.matmul(GA[:], Mt, za[:], start=True, stop=False)
            nc.tensor.matmul(GB[:], Mt, zb[:], start=True, stop=False)
            nc.tensor.matmul(GA[:], Id, e1a[:], start=False, stop=False)
            nc.tensor.matmul(GB[:], Id, e1b[:], start=False, stop=False)
            nc.tensor.matmul(GA[:], R16[b], sta, start=False, stop=True)
            nc.tensor.matmul(GB[:], R16[b], stb, start=False, stop=True)
            gb = gbp.tile([BLK, PW], BF16, tag="gb")
            nc.scalar.copy(gb[:], GB[:])
            if b % 2 == 0:
                og2[p] = outp.tile([BLK, 2, PW], F32, tag="out", name=f"og{p}_{b}")
            og = og2[p]
            nc.vector._custom_dve(WKV_DIV_MUL, out=og[:, b % 2, :], in0=gb[:], in1=GA[:],
                                  s0=_DIV_C0, s1=_DIV_C1, imm2=_DIV_C2)
            if b % 2 == 1:
                r2 = out[(b - 1) * BLK:(b + 1) * BLK, hs]
                nc.scalar.dma_start(r2.rearrange("(s p) h -> p s h", p=BLK), og[:])

        for b in range(NBLK):
            pass1_block(0, b)
        scan_phase(0)
        for p in range(NP - 1):
            for b in range(NBLK):
                pass2_block(p, b)
                pass1_block(p + 1, b)
            scan_phase(p + 1)
        for b in range(NBLK):
            pass2_block(NP - 1, b)

    nc.compile()
    return nc


_cache = {}


def _get_nc():
    if "nc" not in _cache:
        _cache["nc"] = build_nc()
    return _cache["nc"]


def kernel_with_results(key, value, time_decay, time_first, trace=False):
    nc = _get_nc()
    key = np.ascontiguousarray(key, dtype=np.float32)
    value = np.ascontiguousarray(value, dtype=np.float32)
    cT, cD, cS = host_consts(
        np.asarray(time_decay, np.float32), np.asarray(time_first, np.float32)
    )
    in_maps = [
        {"key": key[i], "value": value[i], "cT": cT, "cD": cD, "cS": cS}
        for i in range(N_CORES)
    ]
    res = run_bass_kernel_spmd(nc, in_maps, list(range(N_CORES)), trace=trace)
    outv = np.stack([res.results[i]["out"] for i in range(N_CORES)], axis=0)
    return outv, res


def kernel(key, value, time_decay, time_first):
    outv, _ = kernel_with_results(key, value, time_decay, time_first)
    return outv
